# revision 17
# baseline (speedup 1.0000x reference)
"""Trainium2 Bass kernel for nn_Despawn_65541200937648 (12-level DWT with
sigmoid soft-gate denoising + L1 reg loss).

kernel(**inputs) takes FULL inputs (x [1024,16000] f32, scaling [12,16],
alpha/b_plus/b_minus scalars) and returns (y [1024,16000] f32, reg_loss
scalar f32), matching reference.reference.

Sharding: pure data parallel - rows 1024 -> 8 cores x 128 rows; filters and
thresholds replicated; the scalar reg loss partials are summed on host.

Device strategy: signal positions live on SBUF partitions ("transposed"
layout, built with PE transposes), so each DWT level (conv + down/up-sample)
is a small set of banded-matrix matmuls on the tensor engine (float32r,
1 cycle/row at N>=256). Detail coefficients are gated with 2 ACT sigmoids +
2 DVE ops straight out of PSUM. d_0/d_1 spill to DRAM to fit SBUF.
"""
import os
from contextlib import ExitStack

import numpy as np

import concourse.bass as bass
import concourse.tile as tile
from concourse import bacc, mybir
from concourse.bass_utils import run_bass_kernel_spmd

F32 = mybir.dt.float32
F32R = mybir.dt.float32r
AFT = mybir.ActivationFunctionType
ALU = mybir.AluOpType

LEVELS = 12
M_IN = 16000
L = 16384          # padded signal length
PL = 192           # reflect pad on each side
R = 128            # rows per core
NCORES = 8
DEEP_N = 256       # levels with N < DEEP_N use single-matmul "full" operators
SPILL = (0, 1, 2)  # d levels spilled to DRAM
XCH = 1024         # x DMA chunk columns
CH = 4             # psum chunk: j/u blocks


# ------------------------------------------------------------------ weights
def _qmf(h):
    signs = np.where(np.arange(h.shape[0]) % 2 == 0, 1.0, -1.0).astype(h.dtype)
    return signs * h[::-1]


def _ana_w(h, g):
    """W0, Wm1, Wp1 all [128,128] full-K (edge mats are mostly zero)."""
    W0 = np.zeros((128, 128), dtype=np.float32)
    Wm1 = np.zeros((128, 128), dtype=np.float32)
    Wp1 = np.zeros((128, 128), dtype=np.float32)
    for p in range(128):
        q = p % 64
        f = h if p < 64 else g
        for k in range(16):
            i = 2 * q + k - 7
            if 0 <= i < 128:
                W0[i, p] = f[k]
            elif i < 0:          # block j-1, row 128+i in [121,128)
                Wm1[128 + i, p] = f[k]
            else:                # block j+1, row i-128 in [0,7)
                Wp1[i - 128, p] = f[k]
    return W0, Wm1, Wp1


def _ana_full_w(h, g, N):
    """Deep-level operator [N, 128]: approx at out partitions [0,Nh),
    detail at [64, 64+Nh) (PSUM reads need 32-aligned partition bases)."""
    Nh = N // 2
    W = np.zeros((N, 128), dtype=np.float32)
    for p in range(128):
        if p < Nh:
            q, f = p, h
        elif 64 <= p < 64 + Nh:
            q, f = p - 64, g
        else:
            continue
        for k in range(16):
            i = 2 * q + k - 7
            if 0 <= i < N:
                W[i, p] = f[k]
    return W


def _syn_w(f):
    """Wmain[eps] [128,128]; Wedge [128,128]: rows 96..127 = prev-block taps
    (even u), rows 0..31 = next-block taps (odd u)."""
    Wmain = [np.zeros((128, 128), dtype=np.float32) for _ in range(2)]
    for eps in range(2):
        for t in range(128):
            for i in range(128):
                k = 128 * eps + t + 7 - 2 * i
                if 0 <= k < 16:
                    Wmain[eps][i, t] = f[k]
    Wpe = np.zeros((128, 128), dtype=np.float32)
    Wpo = np.zeros((128, 128), dtype=np.float32)
    for i in range(96, 128):
        for t in range(128):
            k = t + 263 - 2 * i
            if 0 <= k < 16:
                Wpe[i, t] = f[k]
    for i in range(0, 32):
        for t in range(128):
            k = t - 121 - 2 * i
            if 0 <= k < 16:
                Wpo[i, t] = f[k]
    return Wmain, Wpe, Wpo


def _syn_full_w(f, N):
    Nh = N // 2
    W = np.zeros((Nh, N), dtype=np.float32)
    for t in range(N):
        for m in range(Nh):
            k = t + 7 - 2 * m
            if 0 <= k < 16:
                W[m, t] = f[k]
    return W


def build_weights(scaling):
    cols = []
    directory = {}

    def add(name, arr):
        p, w = arr.shape
        if p < 128:
            arr = np.pad(arr, ((0, 128 - p), (0, 0)))
        directory[name] = (sum(c.shape[1] for c in cols), w)
        cols.append(np.ascontiguousarray(arr, dtype=np.float32))

    for lvl in range(LEVELS):
        N = L >> lvl
        h = np.asarray(scaling[lvl], dtype=np.float32)
        g = _qmf(h)
        if N >= DEEP_N:
            W0, Wm1, Wp1 = _ana_w(h, g)
            add(f"a{lvl}_W0", W0)
            add(f"a{lvl}_Wm1", Wm1)
            add(f"a{lvl}_Wp1", Wp1)
            WmA, WpeA, WpoA = _syn_w(h)
            WmD, WpeD, WpoD = _syn_w(g)
            add(f"s{lvl}_Wa_e0", WmA[0]); add(f"s{lvl}_Wa_e1", WmA[1])
            add(f"s{lvl}_Wa_pe", WpeA); add(f"s{lvl}_Wa_po", WpoA)
            add(f"s{lvl}_Wd_e0", WmD[0]); add(f"s{lvl}_Wd_e1", WmD[1])
            add(f"s{lvl}_Wd_pe", WpeD); add(f"s{lvl}_Wd_po", WpoD)
        else:
            add(f"a{lvl}_full", _ana_full_w(h, g, N))
            add(f"s{lvl}_Wa", _syn_full_w(h, N))
            add(f"s{lvl}_Wd", _syn_full_w(g, N))
    add("ones", np.ones((128, 1), dtype=np.float32))
    img = np.concatenate(cols, axis=1)
    return img, directory


# ------------------------------------------------------------------ device IR
def _emit(ctx, nc, tc, wdir, x_in, w_in, scal_in, y_out, reg_out, spills):
    wcols = w_in.shape[1]
    sb = ctx.enter_context(tc.tile_pool(name="sb", bufs=1))
    big = ctx.enter_context(tc.tile_pool(name="big", bufs=1))   # xt / d1_t / d0_t
    xpool = ctx.enter_context(tc.tile_pool(name="xchunk", bufs=2))
    ypool = ctx.enter_context(tc.tile_pool(name="ychunk", bufs=2))
    dspool = ctx.enter_context(tc.tile_pool(name="dsc", bufs=2))
    gpool = ctx.enter_context(tc.tile_pool(name="gate", bufs=2))
    ps_t = ctx.enter_context(tc.tile_pool(name="ps_t", bufs=2, space="PSUM"))
    ps_a = ctx.enter_context(tc.tile_pool(name="ps_a", bufs=3, space="PSUM"))
    ps_s = ctx.enter_context(tc.tile_pool(name="ps_s", bufs=3, space="PSUM"))

    wimg = sb.tile([128, wcols], F32R)
    nc.sync.dma_start(wimg[:], w_in[:])

    def W(name, prange=None):
        off, wd = wdir[name]
        if prange is not None:
            return wimg[prange[0]:prange[1], off:off + wd]
        return wimg[:, off:off + wd]

    # runtime scalars broadcast per partition
    scal128 = sb.tile([128, 3], F32)
    nc.gpsimd.dma_start(scal128[:], scal_in[:].to_broadcast([128, 3]))
    alpha_ap = scal128[:, 0:1]
    nbias_p = sb.tile([128, 1], F32)
    nbias_m = sb.tile([128, 1], F32)
    nalpha = sb.tile([128, 1], F32)
    nc.vector.tensor_tensor(nbias_p[:], scal128[:, 0:1], scal128[:, 1:2], ALU.mult)
    nc.vector.tensor_scalar_mul(nbias_p[:], nbias_p[:], -1.0)
    nc.vector.tensor_tensor(nbias_m[:], scal128[:, 0:1], scal128[:, 2:3], ALU.mult)
    nc.vector.tensor_scalar_mul(nbias_m[:], nbias_m[:], -1.0)
    nc.vector.tensor_scalar_mul(nalpha[:], scal128[:, 0:1], -1.0)

    # identity for PE transposes
    iota_p = sb.tile([128, 1], F32)
    nc.gpsimd.iota(iota_p[:], pattern=[[0, 1]], base=0, channel_multiplier=1,
                   allow_small_or_imprecise_dtypes=True)
    iota_f = sb.tile([128, 128], F32)
    nc.gpsimd.iota(iota_f[:], pattern=[[1, 128]], base=0, channel_multiplier=0,
                   allow_small_or_imprecise_dtypes=True)
    ident = sb.tile([128, 128], F32R)
    nc.vector.tensor_scalar(ident[:], iota_f[:], iota_p[:], None, ALU.is_equal)

    # reg partials
    NPART = 96
    partials = sb.tile([128, NPART], F32)
    nc.vector.memset(partials[:], 0.0)
    npart = [0]

    def add_reduce(src_ap):
        col = npart[0]
        assert col < NPART
        P = src_ap.partition_size()
        nc.vector.tensor_reduce(partials[0:P, col:col + 1], src_ap,
                                axis=mybir.AxisListType.X, op=ALU.add,
                                apply_absolute_value=True)
        npart[0] += 1

    # persistent coefficient tiles (solo blocked layout, zero guard blocks)
    solo = {}
    for lvl in range(1, LEVELS + 1):
        N = L >> lvl
        if N >= 128:
            solo[lvl] = sb.tile([128, (N // 128 + 2) * R], F32R, tag=f"solo{lvl}", name=f"solo{lvl}")
        else:
            solo[lvl] = sb.tile([128, R], F32R, tag=f"solo{lvl}", name=f"solo{lvl}")
    d_tiles = {}
    for lvl in range(LEVELS):
        if lvl in SPILL:
            continue
        N = L >> (lvl + 1)
        if N >= 128:
            d_tiles[lvl] = sb.tile([128, (N // 128 + 2) * R], F32R, tag=f"d{lvl}", name=f"dt{lvl}")
        else:
            d_tiles[lvl] = sb.tile([128, R], F32R, tag=f"d{lvl}", name=f"dt{lvl}")

    def zero_guards(t, nb):
        nc.vector.memset(t[:, 0:R].bitcast(F32), 0.0)
        nc.vector.memset(t[:, (nb + 1) * R:(nb + 2) * R].bitcast(F32), 0.0)

    for lvl in range(1, LEVELS + 1):
        N = L >> lvl
        if N >= 128:
            zero_guards(solo[lvl], N // 128)
    for lvl, t in d_tiles.items():
        N = L >> (lvl + 1)
        if N >= 128:
            zero_guards(t, N // 128)

    def gate_sum(psum_ap, cols):
        """sigmoid(a*(c-b+)) + sigmoid(-a*(c+b-)) for PSUM c. -> SBUF [P, cols]"""
        P = psum_ap.partition_size()
        t1 = gpool.tile([128, CH * R], F32, tag="gt1", name="gt1")[0:P, 0:cols]
        t2 = gpool.tile([128, CH * R], F32, tag="gt2", name="gt2")[0:P, 0:cols]
        nc.scalar.activation(t1, psum_ap, AFT.Sigmoid,
                             bias=nbias_p[0:P], scale=alpha_ap[0:P])
        nc.scalar.activation(t2, psum_ap, AFT.Sigmoid,
                             bias=nbias_m[0:P], scale=nalpha[0:P])
        nc.vector.tensor_tensor(t1, t1, t2, ALU.add)
        return t1

    # ---------------------------------------------------------------- level 0
    KPHASE0 = int(os.environ.get("KPHASE", "5"))
    if KPHASE0 < 1:
        zn0 = ypool.tile([128, CH * R], F32, tag="yn", name="yn")
        nc.vector.memset(zn0[:], 0.0)
        for c0 in range(0, M_IN, 512):
            w = min(512, M_IN - c0)
            nc.sync.dma_start(y_out[:, c0:c0 + w], zn0[:, 0:w])
        rs0 = sb.tile([1, 1], F32)
        nc.vector.memset(rs0[:], 0.0)
        nc.sync.dma_start(reg_out[:], rs0[:])
        return
    xt = big.tile([128, 66 * R], F32R, tag="big66", name="xt")   # 64 data blocks + guards
    a1 = solo[1]
    a1_3 = a1[:].rearrange("p (b r) -> p b r", r=R)

    for half in range(2):
        base_blk = 64 * half
        for ci in range(8192 // XCH):
            pc0 = 8192 * half + XCH * ci
            xc = xpool.tile([128, XCH], F32R, tag="xc", name="xc")
            lo, hi = pc0, pc0 + XCH
            dlo, dhi = max(lo, PL), min(hi, PL + M_IN)
            nc.sync.dma_start(xc[:, dlo - lo:dhi - lo],
                              x_in[:, dlo - PL:dhi - PL])
            if lo < PL:
                # x_pad[P] = x[192-P]: src (padded) = 384-P, within this chunk
                n = PL - lo
                s0 = 384 - 2 * lo
                nc.vector.tensor_copy(xc[:, 0:n], xc[:, s0:s0 - n:-1])
            if hi > PL + M_IN:
                # x_pad[16192+j] = x[15998-j]: src (padded) = 32382-lo-jj
                n = hi - (PL + M_IN)
                j0 = (PL + M_IN) - lo
                s0 = 32382 - 2 * lo - j0
                nc.vector.tensor_copy(xc[:, j0:j0 + n], xc[:, s0:s0 - n:-1])
            for tb in range(0, XCH // R, 4):
                pt = ps_t.tile([128, 4 * R], F32R, tag="pst", name="pst")
                for s in range(4):
                    nc.tensor.transpose(pt[:, s * R:(s + 1) * R],
                                        xc[:, (tb + s) * R:(tb + s + 1) * R],
                                        ident[:])
                db = 1 + ci * (XCH // R) + tb
                nc.scalar.copy(xt[:, db * R:(db + 4) * R], pt[:])
        # seam guard blocks
        gsrc = 64 if half == 0 else 63
        xg = xpool.tile([128, R], F32R, tag="xg", name="xg")
        nc.sync.dma_start(xg[:], x_in[:, gsrc * 128 - PL:gsrc * 128 - PL + R])
        ptg = ps_t.tile([128, 4 * R], F32R, tag="pst", name="pst")
        nc.tensor.transpose(ptg[:, 0:R], xg[:], ident[:])
        if half == 0:
            nc.vector.memset(xt[:, 0:R].bitcast(F32), 0.0)
            nc.scalar.copy(xt[:, 65 * R:66 * R], ptg[:, 0:R])
        else:
            nc.scalar.copy(xt[:, 0:R], ptg[:, 0:R])
            nc.vector.memset(xt[:, 65 * R:66 * R].bitcast(F32), 0.0)

        # analysis level 0 on this half
        for c0 in range(0, 64, CH):
            pa = ps_a.tile([128, CH * R], F32, tag="psa", name="psa")
            nc.tensor.matmul(pa[:], W("a0_W0"),
                             xt[:, (1 + c0) * R:(1 + c0 + CH) * R],
                             start=True, stop=False)
            nc.tensor.matmul(pa[:], W("a0_Wm1"),
                             xt[:, c0 * R:(c0 + CH) * R],
                             start=False, stop=False)
            nc.tensor.matmul(pa[:], W("a0_Wp1"),
                             xt[:, (2 + c0) * R:(2 + c0 + CH) * R],
                             start=False, stop=True)
            pa3 = pa[:].rearrange("p (j r) -> p j r", r=R)
            jg0 = base_blk + c0
            for par in range(2):
                w0 = (jg0 + par) // 2
                prow = 64 * ((jg0 + par) % 2)
                dst = a1_3[prow:prow + 64, 1 + w0:1 + w0 + CH // 2, :]
                src = pa3[0:64, par:CH:2, :]
                if par == 0:
                    nc.vector.tensor_copy(dst, src)
                else:
                    nc.scalar.copy(dst, src)
            gs = gate_sum(pa[64:128, :], CH * R)
            dsc = dspool.tile([64, CH * R], F32R, tag="dsc", name="dsc")
            nc.vector.tensor_tensor(dsc[:], gs, pa[64:128, :], ALU.mult)
            nc.sync.dma_start(spills[0][:, (base_blk + c0) * R:(base_blk + c0 + CH) * R],
                              dsc[:])

    # ------------------------------------------------------------- levels 1+
    def ana_blocked_level(lvl):
        N = L >> lvl
        nb = N // 128
        src = solo[lvl]
        a_next = solo[lvl + 1]
        an3 = a_next[:].rearrange("p (b r) -> p b r", r=R)
        spilled = lvl in SPILL
        if not spilled:
            dt3 = d_tiles[lvl][:].rearrange("p (b r) -> p b r", r=R)
        ch = min(CH, nb)
        for c0 in range(0, nb, ch):
            pa = ps_a.tile([128, CH * R], F32, tag="psa", name="psa")[:, 0:ch * R]
            nc.tensor.matmul(pa, W(f"a{lvl}_W0"),
                             src[:, (1 + c0) * R:(1 + c0 + ch) * R],
                             start=True, stop=False)
            nc.tensor.matmul(pa, W(f"a{lvl}_Wm1"),
                             src[:, c0 * R:(c0 + ch) * R],
                             start=False, stop=False)
            nc.tensor.matmul(pa, W(f"a{lvl}_Wp1"),
                             src[:, (2 + c0) * R:(2 + c0 + ch) * R],
                             start=False, stop=True)
            pa3 = pa.rearrange("p (j r) -> p j r", r=R)
            for par in range(min(2, ch)):
                w0 = (c0 + par) // 2
                prow = 64 * ((c0 + par) % 2)
                njs = (ch - par + 1) // 2
                dst = an3[prow:prow + 64, 1 + w0:1 + w0 + njs, :]
                src_ap = pa3[0:64, par:ch:2, :]
                if par == 0:
                    nc.vector.tensor_copy(dst, src_ap)
                else:
                    nc.scalar.copy(dst, src_ap)
            gs = gate_sum(pa[64:128, :], ch * R)
            if spilled:
                dsc = dspool.tile([64, CH * R], F32R, tag="dsc", name="dsc")[:, 0:ch * R]
                nc.vector.tensor_tensor(dsc, gs, pa[64:128, :], ALU.mult)
                nc.sync.dma_start(spills[lvl][:, c0 * R:(c0 + ch) * R], dsc)
            else:
                gs3 = gs.rearrange("p (j r) -> p j r", r=R)
                for par in range(min(2, ch)):
                    w0 = (c0 + par) // 2
                    prow = 64 * ((c0 + par) % 2)
                    njs = (ch - par + 1) // 2
                    nc.vector.tensor_tensor(
                        dt3[prow:prow + 64, 1 + w0:1 + w0 + njs, :],
                        gs3[0:64, par:ch:2, :],
                        pa3[64:128, par:ch:2, :], ALU.mult)
        if not spilled:
            add_reduce(d_tiles[lvl][:])

    def ana_deep_level(lvl):
        N = L >> lvl
        Nh = N // 2
        rhs = solo[lvl][:, R:2 * R] if N == 128 else solo[lvl][0:N, :]
        pa = ps_a.tile([128, CH * R], F32, tag="psa", name="psa")[0:128, 0:R]
        nc.tensor.matmul(pa, W(f"a{lvl}_full", (0, N)), rhs, start=True, stop=True)
        a_next = solo[lvl + 1]
        dst = a_next[:, R:2 * R] if Nh == 128 else a_next[0:Nh, :]
        if lvl + 1 == LEVELS:
            gs = gate_sum(pa[0:Nh, :], R)
            nc.vector.tensor_tensor(dst, gs, pa[0:Nh, :], ALU.mult)
            add_reduce(dst)
        else:
            nc.scalar.copy(dst, pa[0:Nh, :])
        d_t = d_tiles[lvl]
        ddst = d_t[:, R:2 * R] if Nh == 128 else d_t[0:Nh, :]
        gs = gate_sum(pa[64:64 + Nh, :], R)
        nc.vector.tensor_tensor(ddst, gs, pa[64:64 + Nh, :], ALU.mult)
        add_reduce(ddst)

    KPHASE = int(os.environ.get("KPHASE", "5"))

    def finish_dummy():
        zn = ypool.tile([128, CH * R], F32, tag="yn", name="yn")
        nc.vector.memset(zn[:], 0.0)
        for c0 in range(0, M_IN, 512):
            w = min(512, M_IN - c0)
            nc.sync.dma_start(y_out[:, c0:c0 + w], zn[:, 0:w])
        rs = sb.tile([1, 1], F32)
        nc.vector.memset(rs[:], 0.0)
        nc.sync.dma_start(reg_out[:], rs[:])

    deep_start = next(l for l in range(LEVELS) if (L >> l) < DEEP_N)
    if KPHASE < 2:
        finish_dummy()
        return
    for lvl in range(1, deep_start):
        ana_blocked_level(lvl)
    for lvl in range(deep_start, LEVELS):
        ana_deep_level(lvl)

    # --------------------------------------------------------------- synthesis
    def readback_spill(lvl):
        """DRAM spill (mixed layout) -> solo blocked tile with guards."""
        N = L >> (lvl + 1)        # d_lvl length
        nb = N // 128
        t = big.tile([128, (64 + 2) * R], F32R, tag="big66", name="dspt")
        zero_guards(t, nb)
        sp3 = spills[lvl][:].rearrange("p (j r) -> p j r", r=R)
        t3 = t[:].rearrange("p (b r) -> p b r", r=R)
        nc.sync.dma_start(t3[0:64, 1:1 + nb, :], sp3[:, 0:2 * nb:2, :])
        nc.sync.dma_start(t3[64:128, 1:1 + nb, :], sp3[:, 1:2 * nb:2, :])
        add_reduce(t[:, 0:(nb + 2) * R])
        return t

    def syn_deep_level(lvl):
        N = L >> lvl
        Nh = N // 2
        a_src = solo[lvl + 1]
        a_rhs = a_src[:, R:2 * R] if Nh == 128 else a_src[0:Nh, :]
        d_t = d_tiles[lvl]
        d_rhs = d_t[:, R:2 * R] if Nh == 128 else d_t[0:Nh, :]
        py = ps_s.tile([128, CH * R], F32, tag="pss", name="pss")[0:N, 0:R]
        nc.tensor.matmul(py, W(f"s{lvl}_Wa", (0, Nh)), a_rhs, start=True, stop=False)
        nc.tensor.matmul(py, W(f"s{lvl}_Wd", (0, Nh)), d_rhs, start=False, stop=True)
        dst = solo[lvl][:, R:2 * R] if N == 128 else solo[lvl][0:N, :]
        nc.scalar.copy(dst, py)

    def syn_blocked_level(lvl, d_t, out_cb=None):
        N = L >> lvl
        nby = N // 128
        a_t = solo[lvl + 1]
        a3 = a_t[:].rearrange("p (b r) -> p b r", r=R)
        d3 = d_t[:].rearrange("p (b r) -> p b r", r=R)
        ch = min(CH, nby)
        for u0 in range(0, nby, ch):
            w0 = u0 // 2
            nwb = max(ch // 2, 1)
            py = ps_s.tile([128, CH * R], F32, tag="pss", name="pss")[:, 0:ch * R]
            py3 = py.rearrange("p (u r) -> p u r", r=R)
            ev = py3[:, 0:ch:2, :]
            od = py3[:, 1:ch:2, :]
            # even-u region first (single start=True clears the bank)
            nc.tensor.matmul(ev, W(f"s{lvl}_Wa_e0"), a3[:, 1 + w0:1 + w0 + nwb, :],
                             start=True, stop=False)
            nc.tensor.matmul(ev, W(f"s{lvl}_Wa_pe"),
                             a3[:, w0:w0 + nwb, :],
                             start=False, stop=False)
            nc.tensor.matmul(ev, W(f"s{lvl}_Wd_e0"), d3[:, 1 + w0:1 + w0 + nwb, :],
                             start=False, stop=False)
            nc.tensor.matmul(ev, W(f"s{lvl}_Wd_pe"),
                             d3[:, w0:w0 + nwb, :],
                             start=False, stop=False)
            if ch > 1:
                nc.tensor.matmul(od, W(f"s{lvl}_Wa_e1"),
                                 a3[:, 1 + w0:1 + w0 + nwb, :],
                                 start=False, stop=False)
                nc.tensor.matmul(od, W(f"s{lvl}_Wa_po"),
                                 a3[:, 2 + w0:2 + w0 + nwb, :],
                                 start=False, stop=False)
                nc.tensor.matmul(od, W(f"s{lvl}_Wd_e1"),
                                 a3[:, 1 + w0:1 + w0 + nwb, :] if False else d3[:, 1 + w0:1 + w0 + nwb, :],
                                 start=False, stop=False)
                nc.tensor.matmul(od, W(f"s{lvl}_Wd_po"),
                                 d3[:, 2 + w0:2 + w0 + nwb, :],
                                 start=False, stop=True)
            else:
                # close the group on the last even matmul instead
                pass
            if out_cb is None:
                nc.scalar.copy(solo[lvl][:, (1 + u0) * R:(1 + u0 + ch) * R], py)
            else:
                out_cb(py, u0, ch)

    if KPHASE < 3:
        finish_dummy()
        return
    for lvl in range(LEVELS - 1, deep_start - 1, -1):
        syn_deep_level(lvl)
    for lvl in range(deep_start - 1, 0, -1):
        d_t = d_tiles.get(lvl)
        if d_t is None:
            d_t = readback_spill(lvl)
        syn_blocked_level(lvl, d_t)
    if KPHASE < 5:
        finish_dummy()
        return
    d0_t = readback_spill(0)

    def out_cb(py, u0, ch):
        yt = ypool.tile([128, CH * R], F32R, tag="yt", name="yt")[:, 0:ch * R]
        nc.vector.tensor_copy(yt, py)
        pt = ps_t.tile([128, 4 * R], F32R, tag="pst", name="pst")[:, 0:ch * R]
        for s in range(ch):
            nc.tensor.transpose(pt[:, s * R:(s + 1) * R],
                                yt[:, s * R:(s + 1) * R], ident[:])
        yn = ypool.tile([128, CH * R], F32, tag="yn", name="yn")[:, 0:ch * R]
        nc.scalar.copy(yn, pt)
        p0, p1 = 128 * u0, 128 * (u0 + ch)
        y0, y1 = max(p0, PL) - PL, min(p1, PL + M_IN) - PL
        if y1 > y0:
            nc.sync.dma_start(y_out[:, y0:y1], yn[:, y0 + PL - p0:y1 + PL - p0])

    syn_blocked_level(0, d0_t, out_cb=out_cb)

    # reg scalar: partials -> row sums -> ones.T @ rsum -> [1,1]
    rsum = sb.tile([128, 1], F32)
    nc.vector.tensor_reduce(rsum[:], partials[:], axis=mybir.AxisListType.X,
                            op=ALU.add)
    ones_f = sb.tile([128, 1], F32)
    nc.vector.memset(ones_f[:], 1.0)
    preg = ps_a.tile([128, CH * R], F32, tag="psa", name="psa")[0:1, 0:1]
    nc.tensor.matmul(preg, ones_f[:], rsum[:], start=True, stop=True)
    sreg = sb.tile([1, 1], F32)
    nc.scalar.copy(sreg[:], preg)
    nc.sync.dma_start(reg_out[:], sreg[:])


_CACHE = {}


def _build_nc(wdir, wcols):
    nc = bacc.Bacc("TRN2", target_bir_lowering=False, debug=False)
    x_in = nc.dram_tensor("x", [R, M_IN], F32R, kind="ExternalInput")
    w_in = nc.dram_tensor("wimg", [128, wcols], F32R, kind="ExternalInput")
    scal_in = nc.dram_tensor("svec", [1, 3], F32, kind="ExternalInput")
    y_out = nc.dram_tensor("y", [R, M_IN], F32, kind="ExternalOutput")
    reg_out = nc.dram_tensor("reg", [1, 1], F32, kind="ExternalOutput")
    spills = {lvl: nc.dram_tensor(f"d{lvl}_spill", [64, (L >> (lvl + 1)) // 64 * R], F32R)
              for lvl in SPILL}
    with tile.TileContext(nc) as tc:
        with ExitStack() as ctx:
            _emit(ctx, nc, tc, wdir, x_in, w_in, scal_in, y_out, reg_out, spills)
    nc.compile()
    return nc


def kernel(x, scaling, alpha, b_plus, b_minus):
    x = np.ascontiguousarray(np.asarray(x, dtype=np.float32))
    scaling = np.asarray(scaling, dtype=np.float32)
    wimg, wdir = build_weights(scaling)
    svec = np.array([[float(alpha), float(b_plus), float(b_minus)]],
                    dtype=np.float32)
    key = wimg.shape[1]
    if key not in _CACHE:
        _CACHE[key] = _build_nc(wdir, wimg.shape[1])
    nc = _CACHE[key]
    in_maps = [{"x": np.ascontiguousarray(x[c * R:(c + 1) * R]),
                "wimg": wimg, "svec": svec} for c in range(NCORES)]
    res = run_bass_kernel_spmd(nc, in_maps, core_ids=list(range(NCORES)))
    y = np.concatenate([res.results[c]["y"] for c in range(NCORES)], axis=0)
    reg = np.float32(sum(float(res.results[c]["reg"][0, 0])
                         for c in range(NCORES)))
    return y, reg


# revision 18
# speedup vs baseline: 1.1790x; 1.1790x over previous
"""Trainium2 Bass kernel for nn_Despawn_65541200937648 (12-level DWT with
sigmoid soft-gate denoising + L1 reg loss).

kernel(**inputs) takes FULL inputs (x [1024,16000] f32, scaling [12,16],
alpha/b_plus/b_minus scalars) and returns (y [1024,16000] f32, reg_loss
scalar f32), matching reference.reference.

Sharding: pure data parallel - rows 1024 -> 8 cores x 128 rows; filters and
thresholds replicated; the scalar reg loss partials are summed on host.

Device strategy: signal positions live on SBUF partitions ("transposed"
layout, built with PE transposes), so each DWT level (conv + down/up-sample)
is a small set of banded-matrix matmuls on the tensor engine (float32r,
1 cycle/row at N>=256). Detail coefficients are gated with 2 ACT sigmoids +
2 DVE ops straight out of PSUM. d_0/d_1 spill to DRAM to fit SBUF.
"""
import os
from contextlib import ExitStack

import numpy as np

import concourse.bass as bass
import concourse.tile as tile
from concourse import bacc, mybir
from concourse.bass_utils import run_bass_kernel_spmd

F32 = mybir.dt.float32
F32R = mybir.dt.float32r
AFT = mybir.ActivationFunctionType
ALU = mybir.AluOpType

LEVELS = 12
M_IN = 16000
L = 16384          # padded signal length
PL = 192           # reflect pad on each side
R = 128            # rows per core
NCORES = 8
DEEP_N = 256       # levels with N < DEEP_N use single-matmul "full" operators
SPILL = (0, 1, 2)  # d levels spilled to DRAM
XCH = 1024         # x DMA chunk columns
CH = 4             # psum chunk: j/u blocks


# ------------------------------------------------------------------ weights
def _qmf(h):
    signs = np.where(np.arange(h.shape[0]) % 2 == 0, 1.0, -1.0).astype(h.dtype)
    return signs * h[::-1]


def _ana_w(h, g):
    """W0, Wm1, Wp1 all [128,128] full-K (edge mats are mostly zero)."""
    W0 = np.zeros((128, 128), dtype=np.float32)
    Wm1 = np.zeros((128, 128), dtype=np.float32)
    Wp1 = np.zeros((128, 128), dtype=np.float32)
    for p in range(128):
        q = p % 64
        f = h if p < 64 else g
        for k in range(16):
            i = 2 * q + k - 7
            if 0 <= i < 128:
                W0[i, p] = f[k]
            elif i < 0:          # block j-1, row 128+i in [121,128)
                Wm1[128 + i, p] = f[k]
            else:                # block j+1, row i-128 in [0,7)
                Wp1[i - 128, p] = f[k]
    return W0, Wm1, Wp1


def _ana_full_w(h, g, N):
    """Deep-level operator [N, 128]: approx at out partitions [0,Nh),
    detail at [64, 64+Nh) (PSUM reads need 32-aligned partition bases)."""
    Nh = N // 2
    W = np.zeros((N, 128), dtype=np.float32)
    for p in range(128):
        if p < Nh:
            q, f = p, h
        elif 64 <= p < 64 + Nh:
            q, f = p - 64, g
        else:
            continue
        for k in range(16):
            i = 2 * q + k - 7
            if 0 <= i < N:
                W[i, p] = f[k]
    return W


def _syn_w(f):
    """Wmain[eps] [128,128]; Wedge [128,128]: rows 96..127 = prev-block taps
    (even u), rows 0..31 = next-block taps (odd u)."""
    Wmain = [np.zeros((128, 128), dtype=np.float32) for _ in range(2)]
    for eps in range(2):
        for t in range(128):
            for i in range(128):
                k = 128 * eps + t + 7 - 2 * i
                if 0 <= k < 16:
                    Wmain[eps][i, t] = f[k]
    Wpe = np.zeros((128, 128), dtype=np.float32)
    Wpo = np.zeros((128, 128), dtype=np.float32)
    for i in range(96, 128):
        for t in range(128):
            k = t + 263 - 2 * i
            if 0 <= k < 16:
                Wpe[i, t] = f[k]
    for i in range(0, 32):
        for t in range(128):
            k = t - 121 - 2 * i
            if 0 <= k < 16:
                Wpo[i, t] = f[k]
    return Wmain, Wpe, Wpo


def _syn_full_w(f, N):
    Nh = N // 2
    W = np.zeros((Nh, N), dtype=np.float32)
    for t in range(N):
        for m in range(Nh):
            k = t + 7 - 2 * m
            if 0 <= k < 16:
                W[m, t] = f[k]
    return W


def build_weights(scaling):
    cols = []
    directory = {}

    def add(name, arr):
        p, w = arr.shape
        if p < 128:
            arr = np.pad(arr, ((0, 128 - p), (0, 0)))
        directory[name] = (sum(c.shape[1] for c in cols), w)
        cols.append(np.ascontiguousarray(arr, dtype=np.float32))

    for lvl in range(LEVELS):
        N = L >> lvl
        h = np.asarray(scaling[lvl], dtype=np.float32)
        g = _qmf(h)
        if N >= DEEP_N:
            W0, Wm1, Wp1 = _ana_w(h, g)
            add(f"a{lvl}_W0", W0)
            add(f"a{lvl}_Wm1", Wm1)
            add(f"a{lvl}_Wp1", Wp1)
            WmA, WpeA, WpoA = _syn_w(h)
            WmD, WpeD, WpoD = _syn_w(g)
            add(f"s{lvl}_Wa_e0", WmA[0]); add(f"s{lvl}_Wa_e1", WmA[1])
            add(f"s{lvl}_Wa_pe", WpeA); add(f"s{lvl}_Wa_po", WpoA)
            add(f"s{lvl}_Wd_e0", WmD[0]); add(f"s{lvl}_Wd_e1", WmD[1])
            add(f"s{lvl}_Wd_pe", WpeD); add(f"s{lvl}_Wd_po", WpoD)
        else:
            add(f"a{lvl}_full", _ana_full_w(h, g, N))
            add(f"s{lvl}_Wa", _syn_full_w(h, N))
            add(f"s{lvl}_Wd", _syn_full_w(g, N))
    add("ones", np.ones((128, 1), dtype=np.float32))
    img = np.concatenate(cols, axis=1)
    return img, directory


# ------------------------------------------------------------------ device IR
def _emit(ctx, nc, tc, wdir, x_in, w_in, scal_in, y_out, reg_out, spills):
    wcols = w_in.shape[1]
    sb = ctx.enter_context(tc.tile_pool(name="sb", bufs=1))
    big = ctx.enter_context(tc.tile_pool(name="big", bufs=1))   # xt / d1_t / d0_t
    xpool = ctx.enter_context(tc.tile_pool(name="xchunk", bufs=2))
    ypool = ctx.enter_context(tc.tile_pool(name="ychunk", bufs=2))
    dspool = ctx.enter_context(tc.tile_pool(name="dsc", bufs=2))
    gpool = ctx.enter_context(tc.tile_pool(name="gate", bufs=3))
    ps_t = ctx.enter_context(tc.tile_pool(name="ps_t", bufs=2, space="PSUM"))
    ps_a = ctx.enter_context(tc.tile_pool(name="ps_a", bufs=3, space="PSUM"))
    ps_s = ctx.enter_context(tc.tile_pool(name="ps_s", bufs=3, space="PSUM"))

    wimg = sb.tile([128, wcols], F32R)
    nc.sync.dma_start(wimg[:], w_in[:])

    def W(name, prange=None):
        off, wd = wdir[name]
        if prange is not None:
            return wimg[prange[0]:prange[1], off:off + wd]
        return wimg[:, off:off + wd]

    # runtime scalars broadcast per partition
    scal128 = sb.tile([128, 3], F32)
    nc.gpsimd.dma_start(scal128[:], scal_in[:].to_broadcast([128, 3]))
    alpha_ap = scal128[:, 0:1]
    nbias_p = sb.tile([128, 1], F32)
    nbias_m = sb.tile([128, 1], F32)
    nalpha = sb.tile([128, 1], F32)
    nc.vector.tensor_tensor(nbias_p[:], scal128[:, 0:1], scal128[:, 1:2], ALU.mult)
    nc.vector.tensor_scalar_mul(nbias_p[:], nbias_p[:], -1.0)
    nc.vector.tensor_tensor(nbias_m[:], scal128[:, 0:1], scal128[:, 2:3], ALU.mult)
    nc.vector.tensor_scalar_mul(nbias_m[:], nbias_m[:], -1.0)
    nc.vector.tensor_scalar_mul(nalpha[:], scal128[:, 0:1], -1.0)

    # identity for PE transposes
    iota_p = sb.tile([128, 1], F32)
    nc.gpsimd.iota(iota_p[:], pattern=[[0, 1]], base=0, channel_multiplier=1,
                   allow_small_or_imprecise_dtypes=True)
    iota_f = sb.tile([128, 128], F32)
    nc.gpsimd.iota(iota_f[:], pattern=[[1, 128]], base=0, channel_multiplier=0,
                   allow_small_or_imprecise_dtypes=True)
    ident = sb.tile([128, 128], F32R)
    nc.vector.tensor_scalar(ident[:], iota_f[:], iota_p[:], None, ALU.is_equal)

    # reg partials
    NPART = 96
    partials = sb.tile([128, NPART], F32)
    nc.vector.memset(partials[:], 0.0)
    npart = [0]

    def add_reduce(src_ap):
        col = npart[0]
        assert col < NPART
        P = src_ap.partition_size()
        nc.vector.tensor_reduce(partials[0:P, col:col + 1], src_ap,
                                axis=mybir.AxisListType.X, op=ALU.add,
                                apply_absolute_value=True)
        npart[0] += 1

    # persistent coefficient tiles (solo blocked layout, zero guard blocks)
    solo = {}
    for lvl in range(1, LEVELS + 1):
        N = L >> lvl
        if N >= 128:
            solo[lvl] = sb.tile([128, (N // 128 + 2) * R], F32R, tag=f"solo{lvl}", name=f"solo{lvl}")
        else:
            solo[lvl] = sb.tile([128, R], F32R, tag=f"solo{lvl}", name=f"solo{lvl}")
    d_tiles = {}
    for lvl in range(LEVELS):
        if lvl in SPILL:
            continue
        N = L >> (lvl + 1)
        if N >= 128:
            d_tiles[lvl] = sb.tile([128, (N // 128 + 2) * R], F32R, tag=f"d{lvl}", name=f"dt{lvl}")
        else:
            d_tiles[lvl] = sb.tile([128, R], F32R, tag=f"d{lvl}", name=f"dt{lvl}")

    def zero_guards(t, nb):
        nc.vector.memset(t[:, 0:R].bitcast(F32), 0.0)
        nc.vector.memset(t[:, (nb + 1) * R:(nb + 2) * R].bitcast(F32), 0.0)

    for lvl in range(1, LEVELS + 1):
        N = L >> lvl
        if N >= 128:
            zero_guards(solo[lvl], N // 128)
    for lvl, t in d_tiles.items():
        N = L >> (lvl + 1)
        if N >= 128:
            zero_guards(t, N // 128)

    def gate_sum(psum_ap, cols):
        """sigmoid(a*(c-b+)) + sigmoid(-a*(c+b-)) for PSUM c. -> SBUF [P, cols]"""
        P = psum_ap.partition_size()
        t1 = gpool.tile([128, CH * R], F32, tag="gt1", name="gt1")[0:P, 0:cols]
        t2 = gpool.tile([128, CH * R], F32, tag="gt2", name="gt2")[0:P, 0:cols]
        nc.scalar.activation(t1, psum_ap, AFT.Sigmoid,
                             bias=nbias_p[0:P], scale=alpha_ap[0:P])
        nc.scalar.activation(t2, psum_ap, AFT.Sigmoid,
                             bias=nbias_m[0:P], scale=nalpha[0:P])
        nc.gpsimd.tensor_tensor(t1, t1, t2, ALU.add)
        return t1

    # ---------------------------------------------------------------- level 0
    KPHASE0 = int(os.environ.get("KPHASE", "5"))
    if KPHASE0 < 1:
        zn0 = ypool.tile([128, CH * R], F32, tag="yn", name="yn")
        nc.vector.memset(zn0[:], 0.0)
        for c0 in range(0, M_IN, 512):
            w = min(512, M_IN - c0)
            nc.sync.dma_start(y_out[:, c0:c0 + w], zn0[:, 0:w])
        rs0 = sb.tile([1, 1], F32)
        nc.vector.memset(rs0[:], 0.0)
        nc.sync.dma_start(reg_out[:], rs0[:])
        return
    xt = big.tile([128, 66 * R], F32R, tag="big66", name="xt")   # 64 data blocks + guards
    a1 = solo[1]
    a1_3 = a1[:].rearrange("p (b r) -> p b r", r=R)

    for half in range(2):
        base_blk = 64 * half
        for ci in range(8192 // XCH):
            pc0 = 8192 * half + XCH * ci
            xc = xpool.tile([128, XCH], F32R, tag="xc", name="xc")
            lo, hi = pc0, pc0 + XCH
            dlo, dhi = max(lo, PL), min(hi, PL + M_IN)
            nc.sync.dma_start(xc[:, dlo - lo:dhi - lo],
                              x_in[:, dlo - PL:dhi - PL])
            if lo < PL:
                # x_pad[P] = x[192-P]: src (padded) = 384-P, within this chunk
                n = PL - lo
                s0 = 384 - 2 * lo
                nc.vector.tensor_copy(xc[:, 0:n], xc[:, s0:s0 - n:-1])
            if hi > PL + M_IN:
                # x_pad[16192+j] = x[15998-j]: src (padded) = 32382-lo-jj
                n = hi - (PL + M_IN)
                j0 = (PL + M_IN) - lo
                s0 = 32382 - 2 * lo - j0
                nc.vector.tensor_copy(xc[:, j0:j0 + n], xc[:, s0:s0 - n:-1])
            for tb in range(0, XCH // R, 4):
                pt = ps_t.tile([128, 4 * R], F32R, tag="pst", name="pst")
                for s in range(4):
                    nc.tensor.transpose(pt[:, s * R:(s + 1) * R],
                                        xc[:, (tb + s) * R:(tb + s + 1) * R],
                                        ident[:])
                db = 1 + ci * (XCH // R) + tb
                if (tb // 4) % 2 == 0:
                    nc.scalar.copy(xt[:, db * R:(db + 4) * R], pt[:])
                else:
                    nc.vector.tensor_copy(xt[:, db * R:(db + 4) * R], pt[:])
        # seam guard blocks
        gsrc = 64 if half == 0 else 63
        xg = xpool.tile([128, R], F32R, tag="xg", name="xg")
        nc.sync.dma_start(xg[:], x_in[:, gsrc * 128 - PL:gsrc * 128 - PL + R])
        ptg = ps_t.tile([128, 4 * R], F32R, tag="pst", name="pst")
        nc.tensor.transpose(ptg[:, 0:R], xg[:], ident[:])
        if half == 0:
            nc.vector.memset(xt[:, 0:R].bitcast(F32), 0.0)
            nc.scalar.copy(xt[:, 65 * R:66 * R], ptg[:, 0:R])
        else:
            nc.scalar.copy(xt[:, 0:R], ptg[:, 0:R])
            nc.vector.memset(xt[:, 65 * R:66 * R].bitcast(F32), 0.0)

        # analysis level 0 on this half
        for c0 in range(0, 64, CH):
            pa = ps_a.tile([128, CH * R], F32, tag="psa", name="psa")
            nc.tensor.matmul(pa[:], W("a0_W0"),
                             xt[:, (1 + c0) * R:(1 + c0 + CH) * R],
                             start=True, stop=False)
            nc.tensor.matmul(pa[:], W("a0_Wm1"),
                             xt[:, c0 * R:(c0 + CH) * R],
                             start=False, stop=False)
            nc.tensor.matmul(pa[:], W("a0_Wp1"),
                             xt[:, (2 + c0) * R:(2 + c0 + CH) * R],
                             start=False, stop=True)
            pa3 = pa[:].rearrange("p (j r) -> p j r", r=R)
            jg0 = base_blk + c0
            for par in range(2):
                w0 = (jg0 + par) // 2
                prow = 64 * ((jg0 + par) % 2)
                dst = a1_3[prow:prow + 64, 1 + w0:1 + w0 + CH // 2, :]
                src = pa3[0:64, par:CH:2, :]
                if par == 0:
                    nc.vector.tensor_copy(dst, src)
                else:
                    nc.scalar.copy(dst, src)
            gs = gate_sum(pa[64:128, :], CH * R)
            dsc = dspool.tile([64, CH * R], F32R, tag="dsc", name="dsc")
            nc.vector.tensor_tensor(dsc[:], gs, pa[64:128, :], ALU.mult)
            nc.sync.dma_start(spills[0][:, (base_blk + c0) * R:(base_blk + c0 + CH) * R],
                              dsc[:])

    # ------------------------------------------------------------- levels 1+
    def ana_blocked_level(lvl):
        N = L >> lvl
        nb = N // 128
        src = solo[lvl]
        a_next = solo[lvl + 1]
        an3 = a_next[:].rearrange("p (b r) -> p b r", r=R)
        spilled = lvl in SPILL
        if not spilled:
            dt3 = d_tiles[lvl][:].rearrange("p (b r) -> p b r", r=R)
        ch = min(CH, nb)
        for c0 in range(0, nb, ch):
            pa = ps_a.tile([128, CH * R], F32, tag="psa", name="psa")[:, 0:ch * R]
            nc.tensor.matmul(pa, W(f"a{lvl}_W0"),
                             src[:, (1 + c0) * R:(1 + c0 + ch) * R],
                             start=True, stop=False)
            nc.tensor.matmul(pa, W(f"a{lvl}_Wm1"),
                             src[:, c0 * R:(c0 + ch) * R],
                             start=False, stop=False)
            nc.tensor.matmul(pa, W(f"a{lvl}_Wp1"),
                             src[:, (2 + c0) * R:(2 + c0 + ch) * R],
                             start=False, stop=True)
            pa3 = pa.rearrange("p (j r) -> p j r", r=R)
            for par in range(min(2, ch)):
                w0 = (c0 + par) // 2
                prow = 64 * ((c0 + par) % 2)
                njs = (ch - par + 1) // 2
                dst = an3[prow:prow + 64, 1 + w0:1 + w0 + njs, :]
                src_ap = pa3[0:64, par:ch:2, :]
                if par == 0:
                    nc.vector.tensor_copy(dst, src_ap)
                else:
                    nc.scalar.copy(dst, src_ap)
            gs = gate_sum(pa[64:128, :], ch * R)
            if spilled:
                dsc = dspool.tile([64, CH * R], F32R, tag="dsc", name="dsc")[:, 0:ch * R]
                nc.vector.tensor_tensor(dsc, gs, pa[64:128, :], ALU.mult)
                nc.sync.dma_start(spills[lvl][:, c0 * R:(c0 + ch) * R], dsc)
            else:
                gs3 = gs.rearrange("p (j r) -> p j r", r=R)
                for par in range(min(2, ch)):
                    w0 = (c0 + par) // 2
                    prow = 64 * ((c0 + par) % 2)
                    njs = (ch - par + 1) // 2
                    nc.vector.tensor_tensor(
                        dt3[prow:prow + 64, 1 + w0:1 + w0 + njs, :],
                        gs3[0:64, par:ch:2, :],
                        pa3[64:128, par:ch:2, :], ALU.mult)
        if not spilled:
            add_reduce(d_tiles[lvl][:])

    def ana_deep_level(lvl):
        N = L >> lvl
        Nh = N // 2
        rhs = solo[lvl][:, R:2 * R] if N == 128 else solo[lvl][0:N, :]
        pa = ps_a.tile([128, CH * R], F32, tag="psa", name="psa")[0:128, 0:R]
        nc.tensor.matmul(pa, W(f"a{lvl}_full", (0, N)), rhs, start=True, stop=True)
        a_next = solo[lvl + 1]
        dst = a_next[:, R:2 * R] if Nh == 128 else a_next[0:Nh, :]
        if lvl + 1 == LEVELS:
            gs = gate_sum(pa[0:Nh, :], R)
            nc.vector.tensor_tensor(dst, gs, pa[0:Nh, :], ALU.mult)
            add_reduce(dst)
        else:
            nc.scalar.copy(dst, pa[0:Nh, :])
        d_t = d_tiles[lvl]
        ddst = d_t[:, R:2 * R] if Nh == 128 else d_t[0:Nh, :]
        gs = gate_sum(pa[64:64 + Nh, :], R)
        nc.vector.tensor_tensor(ddst, gs, pa[64:64 + Nh, :], ALU.mult)
        add_reduce(ddst)

    KPHASE = int(os.environ.get("KPHASE", "5"))

    def finish_dummy():
        zn = ypool.tile([128, CH * R], F32, tag="yn", name="yn")
        nc.vector.memset(zn[:], 0.0)
        for c0 in range(0, M_IN, 512):
            w = min(512, M_IN - c0)
            nc.sync.dma_start(y_out[:, c0:c0 + w], zn[:, 0:w])
        rs = sb.tile([1, 1], F32)
        nc.vector.memset(rs[:], 0.0)
        nc.sync.dma_start(reg_out[:], rs[:])

    deep_start = next(l for l in range(LEVELS) if (L >> l) < DEEP_N)
    if KPHASE < 2:
        finish_dummy()
        return
    for lvl in range(1, deep_start):
        ana_blocked_level(lvl)
    for lvl in range(deep_start, LEVELS):
        ana_deep_level(lvl)

    # --------------------------------------------------------------- synthesis
    def readback_spill(lvl):
        """DRAM spill (mixed layout) -> solo blocked tile with guards."""
        N = L >> (lvl + 1)        # d_lvl length
        nb = N // 128
        t = big.tile([128, (64 + 2) * R], F32R, tag="big66", name="dspt")
        zero_guards(t, nb)
        sp3 = spills[lvl][:].rearrange("p (j r) -> p j r", r=R)
        t3 = t[:].rearrange("p (b r) -> p b r", r=R)
        nc.sync.dma_start(t3[0:64, 1:1 + nb, :], sp3[:, 0:2 * nb:2, :])
        nc.sync.dma_start(t3[64:128, 1:1 + nb, :], sp3[:, 1:2 * nb:2, :])
        add_reduce(t[:, 0:(nb + 2) * R])
        return t

    def syn_deep_level(lvl):
        N = L >> lvl
        Nh = N // 2
        a_src = solo[lvl + 1]
        a_rhs = a_src[:, R:2 * R] if Nh == 128 else a_src[0:Nh, :]
        d_t = d_tiles[lvl]
        d_rhs = d_t[:, R:2 * R] if Nh == 128 else d_t[0:Nh, :]
        py = ps_s.tile([128, CH * R], F32, tag="pss", name="pss")[0:N, 0:R]
        nc.tensor.matmul(py, W(f"s{lvl}_Wa", (0, Nh)), a_rhs, start=True, stop=False)
        nc.tensor.matmul(py, W(f"s{lvl}_Wd", (0, Nh)), d_rhs, start=False, stop=True)
        dst = solo[lvl][:, R:2 * R] if N == 128 else solo[lvl][0:N, :]
        nc.scalar.copy(dst, py)

    def syn_blocked_level(lvl, d_t, out_cb=None):
        N = L >> lvl
        nby = N // 128
        a_t = solo[lvl + 1]
        a3 = a_t[:].rearrange("p (b r) -> p b r", r=R)
        d3 = d_t[:].rearrange("p (b r) -> p b r", r=R)
        ch = min(CH, nby)
        for u0 in range(0, nby, ch):
            w0 = u0 // 2
            nwb = max(ch // 2, 1)
            py = ps_s.tile([128, CH * R], F32, tag="pss", name="pss")[:, 0:ch * R]
            py3 = py.rearrange("p (u r) -> p u r", r=R)
            ev = py3[:, 0:ch:2, :]
            od = py3[:, 1:ch:2, :]
            # even-u region first (single start=True clears the bank)
            nc.tensor.matmul(ev, W(f"s{lvl}_Wa_e0"), a3[:, 1 + w0:1 + w0 + nwb, :],
                             start=True, stop=False)
            nc.tensor.matmul(ev, W(f"s{lvl}_Wa_pe"),
                             a3[:, w0:w0 + nwb, :],
                             start=False, stop=False)
            nc.tensor.matmul(ev, W(f"s{lvl}_Wd_e0"), d3[:, 1 + w0:1 + w0 + nwb, :],
                             start=False, stop=False)
            nc.tensor.matmul(ev, W(f"s{lvl}_Wd_pe"),
                             d3[:, w0:w0 + nwb, :],
                             start=False, stop=False)
            if ch > 1:
                nc.tensor.matmul(od, W(f"s{lvl}_Wa_e1"),
                                 a3[:, 1 + w0:1 + w0 + nwb, :],
                                 start=False, stop=False)
                nc.tensor.matmul(od, W(f"s{lvl}_Wa_po"),
                                 a3[:, 2 + w0:2 + w0 + nwb, :],
                                 start=False, stop=False)
                nc.tensor.matmul(od, W(f"s{lvl}_Wd_e1"),
                                 a3[:, 1 + w0:1 + w0 + nwb, :] if False else d3[:, 1 + w0:1 + w0 + nwb, :],
                                 start=False, stop=False)
                nc.tensor.matmul(od, W(f"s{lvl}_Wd_po"),
                                 d3[:, 2 + w0:2 + w0 + nwb, :],
                                 start=False, stop=True)
            else:
                # close the group on the last even matmul instead
                pass
            if out_cb is None:
                nc.scalar.copy(solo[lvl][:, (1 + u0) * R:(1 + u0 + ch) * R], py)
            else:
                out_cb(py, u0, ch)

    if KPHASE < 3:
        finish_dummy()
        return
    for lvl in range(LEVELS - 1, deep_start - 1, -1):
        syn_deep_level(lvl)
    for lvl in range(deep_start - 1, 0, -1):
        d_t = d_tiles.get(lvl)
        if d_t is None:
            d_t = readback_spill(lvl)
        syn_blocked_level(lvl, d_t)
    if KPHASE < 5:
        finish_dummy()
        return
    d0_t = readback_spill(0)

    def out_cb(py, u0, ch):
        yt = ypool.tile([128, CH * R], F32R, tag="yt", name="yt")[:, 0:ch * R]
        nc.vector.tensor_copy(yt, py)
        pt = ps_t.tile([128, 4 * R], F32R, tag="pst", name="pst")[:, 0:ch * R]
        for s in range(ch):
            nc.tensor.transpose(pt[:, s * R:(s + 1) * R],
                                yt[:, s * R:(s + 1) * R], ident[:])
        yn = ypool.tile([128, CH * R], F32, tag="yn", name="yn")[:, 0:ch * R]
        nc.scalar.copy(yn, pt)
        p0, p1 = 128 * u0, 128 * (u0 + ch)
        y0, y1 = max(p0, PL) - PL, min(p1, PL + M_IN) - PL
        if y1 > y0:
            nc.sync.dma_start(y_out[:, y0:y1], yn[:, y0 + PL - p0:y1 + PL - p0])

    syn_blocked_level(0, d0_t, out_cb=out_cb)

    # reg scalar: partials -> row sums -> ones.T @ rsum -> [1,1]
    rsum = sb.tile([128, 1], F32)
    nc.vector.tensor_reduce(rsum[:], partials[:], axis=mybir.AxisListType.X,
                            op=ALU.add)
    ones_f = sb.tile([128, 1], F32)
    nc.vector.memset(ones_f[:], 1.0)
    preg = ps_a.tile([128, CH * R], F32, tag="psa", name="psa")[0:1, 0:1]
    nc.tensor.matmul(preg, ones_f[:], rsum[:], start=True, stop=True)
    sreg = sb.tile([1, 1], F32)
    nc.scalar.copy(sreg[:], preg)
    nc.sync.dma_start(reg_out[:], sreg[:])


_CACHE = {}


def _build_nc(wdir, wcols):
    nc = bacc.Bacc("TRN2", target_bir_lowering=False, debug=False)
    x_in = nc.dram_tensor("x", [R, M_IN], F32R, kind="ExternalInput")
    w_in = nc.dram_tensor("wimg", [128, wcols], F32R, kind="ExternalInput")
    scal_in = nc.dram_tensor("svec", [1, 3], F32, kind="ExternalInput")
    y_out = nc.dram_tensor("y", [R, M_IN], F32, kind="ExternalOutput")
    reg_out = nc.dram_tensor("reg", [1, 1], F32, kind="ExternalOutput")
    spills = {lvl: nc.dram_tensor(f"d{lvl}_spill", [64, (L >> (lvl + 1)) // 64 * R], F32R)
              for lvl in SPILL}
    with tile.TileContext(nc) as tc:
        with ExitStack() as ctx:
            _emit(ctx, nc, tc, wdir, x_in, w_in, scal_in, y_out, reg_out, spills)
    nc.compile()
    return nc


def kernel(x, scaling, alpha, b_plus, b_minus):
    x = np.ascontiguousarray(np.asarray(x, dtype=np.float32))
    scaling = np.asarray(scaling, dtype=np.float32)
    wimg, wdir = build_weights(scaling)
    svec = np.array([[float(alpha), float(b_plus), float(b_minus)]],
                    dtype=np.float32)
    key = wimg.shape[1]
    if key not in _CACHE:
        _CACHE[key] = _build_nc(wdir, wimg.shape[1])
    nc = _CACHE[key]
    in_maps = [{"x": np.ascontiguousarray(x[c * R:(c + 1) * R]),
                "wimg": wimg, "svec": svec} for c in range(NCORES)]
    res = run_bass_kernel_spmd(nc, in_maps, core_ids=list(range(NCORES)))
    y = np.concatenate([res.results[c]["y"] for c in range(NCORES)], axis=0)
    reg = np.float32(sum(float(res.results[c]["reg"][0, 0])
                         for c in range(NCORES)))
    return y, reg
